# revision 43
# baseline (speedup 1.0000x reference)
# DenseGATConv on 8 Trainium2 NeuronCores (Bass/Tile, SPMD over destination rows).
#
# Math: h = x@W ; el/er = head-wise <h, att> ; e_ij = leaky(el_i + er_j) ;
#       alpha = softmax_j(mask(e)) ; out_i = sum_j alpha_ij h_j + bias.
# Key identity: exp(leaky(s)) = max(exp(s), exp(0.2 s)) since exp is monotone
# and leaky(s) = max(s, 0.2 s).  With s_ij = el_i + er_j both branches are
# rank-1 outer products, so the masked unnormalized attention splits as
#   pm = m*A + m*relu(B - A),  A = al_i ar_j (rank-1), B = bl_i br_j (rank-1).
# The A-part rides matmuls end to end (P1, flipped aggregation); the residual
# needs one elementwise relu+mask pass over [N, NB, H], done as
# PE rank-2 matmul (4 heads packed into PE row-groups, concurrent) ->
# ScalarE relu (bf16 PSUM) -> DVE mask-mult -> PE aggregation (po2).
# The denominator rides as a ones-column in the aggregation matmuls.
#
# Sharding: destination rows i split across 8 cores (512 rows each); every core
# computes the full h (it needs all source nodes j anyway); params replicated.
# Whole datapath is bf16 on the PE (validated: rel err ~2e-3 vs 2e-2 budget);
# adjacency is cast to bf16 on the host (halves HBM traffic, kills the
# on-device int32->bf16 cast pass).
import numpy as np

N, IN_C, HEADS, OUT_C = 4096, 256, 4, 64
HC = HEADS * OUT_C          # 256
NCORES = 8
NB = N // NCORES            # 512 destination rows per core
JT = N // 128               # 32 source-node tiles
IT = NB // 128              # 4 row subtiles per core
C65 = OUT_C + 1             # head slice + ones column
WC = HC + HEADS             # W cols + War cols

TRACE = False               # test.py flips this to collect HW exec time
LAST_RESULTS = {}           # exec_time_ns etc. stashed here when TRACE

_compiled = {}


def _emit(ctx, tc, nc, io):
    import concourse.bass as bass
    import concourse.masks as masks
    from concourse import mybir

    dt = mybir.dt
    Alu = mybir.AluOpType
    Act = mybir.ActivationFunctionType

    xT, xoT, adjbT, Waug, Wal, bias, out = (
        io["xT"], io["xoT"], io["adjbT"], io["Waug"], io["Wal"],
        io["bias"], io["out"],
    )

    big = ctx.enter_context(tc.tile_pool(name="big", bufs=1))
    work = ctx.enter_context(tc.tile_pool(name="work", bufs=4))
    work2 = ctx.enter_context(tc.tile_pool(name="work2", bufs=4))
    tr = ctx.enter_context(tc.tile_pool(name="tr", bufs=3))

    # ---- constants / params -------------------------------------------------
    idf = big.tile([128, 128], dt.float32, tag="idf")
    masks.make_identity(nc, idf[:])
    idb = big.tile([128, 128], dt.bfloat16, tag="idb")
    masks.make_identity(nc, idb[:])
    bias_b = big.tile([128, HC], dt.float32, tag="bias_b")
    bias_bcast_ap = bass.AP(
        tensor=bias.tensor, offset=bias.offset, ap=[[0, 128]] + list(bias.ap)
    )
    nc.gpsimd.dma_start(out=bias_b[:], in_=bias_bcast_ap)

    # params first (small, needed by the first h-matmul), then x transposed
    # in column chunks so the h-matmul loop can start as soon as the first
    # chunk lands
    waug = []
    wal = []
    xo = []
    for ct in range(2):
        wg = big.tile([128, WC], dt.bfloat16, tag=f"waug{ct}")
        nc.sync.dma_start(out=wg[:], in_=Waug[ct * 128:(ct + 1) * 128, :])
        waug.append(wg)
        wl = big.tile([128, HEADS], dt.bfloat16, tag=f"wal{ct}")
        nc.sync.dma_start(out=wl[:], in_=Wal[ct * 128:(ct + 1) * 128, :])
        wal.append(wl)
        t = big.tile([128, NB], dt.bfloat16, tag=f"xoT{ct}")
        nc.sync.dma_start(out=t[:], in_=xoT[ct * 128:(ct + 1) * 128, :])
        xo.append(t)
    xTr = []
    for ct in range(2):
        xf = big.tile([128, N], dt.bfloat16, tag=f"xTr{ct}")
        xTr.append(xf)
    for c in range(4):
        for ct in range(2):
            eng = nc.sync if c % 2 == 0 else nc.scalar
            eng.dma_start(
                out=xTr[ct][:, c * 1024:(c + 1) * 1024],
                in_=xT[ct * 128:(ct + 1) * 128, c * 1024:(c + 1) * 1024])
    # adjacency: host-pretransposed + pre-cast bf16 [N, NB], plus the packed
    # 16-bit edge masks (int32 pairs) for the bitwise-AND mask multiply.
    # Stored as single wide tiles, loaded with a handful of big multi-tile
    # span DMAs (each dma_start costs ~0.7us of issuing-engine queue time),
    # interleaved across the two hardware DGE queues in consumption order.
    # gpsimd's DMA path is software DGE at ~14 GB/s - bulk data must not
    # ride it.
    adjM = io["adjmask"]
    adjT_big = big.tile([128, JT * NB], dt.bfloat16, tag="adjT_big")
    adjm_big = big.tile([128, JT * NB // 2], dt.int32, tag="adjm_big")
    adjT = [adjT_big[:, jt * NB:(jt + 1) * NB] for jt in range(JT)]
    adjm = [adjm_big[:, jt * NB // 2:(jt + 1) * NB // 2] for jt in range(JT)]

    def adj_span(eng, lo, hi):
        # adjT_big[p, jt*NB + i] = adjbT[jt*128 + p, i] for jt in [lo, hi)
        n = hi - lo
        eng.dma_start(
            out=adjT_big[:, lo * NB:hi * NB]
            .rearrange("p (j i) -> p j i", j=n),
            in_=adjbT[lo * 128:hi * 128, :]
            .rearrange("(j p) i -> p j i", p=128))

    def adjm_span(eng, lo, hi):
        n = hi - lo
        eng.dma_start(
            out=adjm_big[:, lo * NB // 2:hi * NB // 2]
            .rearrange("p (j i) -> p j i", j=n),
            in_=adjM[lo * 128:hi * 128, :]
            .rearrange("(j p) i -> p j i", p=128))

    adj_span(nc.scalar, 0, 8)      # scalar queue: jt 0-7 first
    adj_span(nc.sync, 8, 16)
    adj_span(nc.scalar, 16, 24)
    adj_span(nc.sync, 24, 32)
    adjm_span(nc.scalar, 0, 16)
    adjm_span(nc.sync, 16, 32)

    ht = []
    er_pack = big.tile([128, JT * HEADS], dt.float32, tag="er_pack")
    ar_pack = big.tile([128, JT * HEADS], dt.float32, tag="ar_pack")
    br_pack = big.tile([128, JT * HEADS], dt.float32, tag="br_pack")
    erp = er_pack[:].rearrange("p (h j) -> p h j", h=HEADS)
    arbr = big.tile([128, N], dt.bfloat16, tag="arbr")
    drhs = big.tile([128, NB], dt.bfloat16, tag="drhs")
    # al transposed to destination-row partitions: col it*HEADS+h
    al_colsT = big.tile([128, IT * HEADS], dt.float32, tag="al_colsT")

    with tc.tile_pool(name="ps", bufs=1, space="PSUM") as ps, \
         tc.tile_pool(name="psh", bufs=2, space="PSUM") as psh:
        # PE warmup during the initial DMA window: keeps the HAM activity
        # monitor busy so the 2.4 GHz clock is up before real matmuls start.
        warm = ps.tile([128, 128], dt.float32, tag="warm")
        for _ in range(56):
            nc.tensor.matmul(warm[:, 0:64], lhsT=idb[:], rhs=idb[:, 0:64],
                             start=True, stop=True)

        # ---- h65 (bf16 h + ones col) and er via one augmented matmul --------
        for nt in range(JT):
            hps = psh.tile([128, WC], dt.float32, tag="hps")
            for ct in range(2):
                nc.tensor.matmul(
                    hps[:], lhsT=xTr[ct][:, nt * 128:(nt + 1) * 128],
                    rhs=waug[ct][:], start=(ct == 0), stop=(ct == 1),
                )
            t = big.tile([128, HEADS * C65], dt.bfloat16, tag=f"h65_{nt}")
            hr = t[:].rearrange("p (h c) -> p h c", c=C65)
            hpr = hps[:, 0:HC].rearrange("p (h c) -> p h c", c=OUT_C)
            if nt % 2 == 0:
                nc.scalar.copy(hr[:, :, 0:OUT_C], hpr[:, :, :])
            else:
                nc.vector.tensor_copy(hr[:, :, 0:OUT_C], hpr[:, :, :])
            nc.vector.memset(hr[:, :, OUT_C], 1.0)
            nc.vector.tensor_copy(erp[:, :, nt], hps[:, HC:WC])
            ht.append(t)
        for jt in range(JT):
            nc.scalar.dma_start(out=adjm[jt][:],
                                in_=adjM[jt * 128:(jt + 1) * 128, :])

        # ---- exp(er) rows; pack per-head [br; -ar] at PE row-group bases ----
        nc.scalar.activation(ar_pack[:], er_pack[:], Act.Exp)
        nc.scalar.activation(br_pack[:], er_pack[:], Act.Exp, scale=0.2)
        arb16 = big.tile([128, JT * HEADS], dt.bfloat16, tag="arb16")
        brb16 = big.tile([128, JT * HEADS], dt.bfloat16, tag="brb16")
        # negate ar here so the d-matmul computes B - A with positive al rhs
        nc.vector.tensor_scalar_mul(arb16[:], ar_pack[:], -1.0)
        nc.vector.tensor_copy(brb16[:], br_pack[:])
        arT_ps = ps.tile([128, 128], dt.bfloat16, tag="arT")
        brT_ps = ps.tile([128, 128], dt.bfloat16, tag="brT")
        nc.tensor.transpose(arT_ps[:], arb16[:], idb[:])
        nc.tensor.transpose(brT_ps[:], brb16[:], idb[:])
        arT_sb = big.tile([128, 128], dt.bfloat16, tag="arT_sb")
        brT_sb = big.tile([128, 128], dt.bfloat16, tag="brT_sb")
        nc.vector.tensor_copy(arT_sb[:], arT_ps[:])
        nc.vector.tensor_copy(brT_sb[:], brT_ps[:])
        # arbr rows (partition-packed): 32h = br_h, 32h+1 = -ar_h -> four
        # concurrent PE row-group matmuls (one per head), each writing its
        # own PSUM bank.  Tiny SBUF-SBUF moves ride the (software) gpsimd
        # DMA queue, keeping the hardware queues free for bulk data.
        for h in range(HEADS):
            nc.gpsimd.dma_start(
                out=arbr[32 * h:32 * h + 1, :],
                in_=brT_sb[h * JT:(h + 1) * JT, :])
            nc.gpsimd.dma_start(
                out=arbr[32 * h + 1:32 * h + 2, :],
                in_=arT_sb[h * JT:(h + 1) * JT, :])

        # ---- el side: one [4, NB] matmul, exp, block-diagonal d rhs ---------
        elp = ps.tile([HEADS, NB], dt.float32, tag="elp")
        for ct in range(2):
            nc.tensor.matmul(elp[:], lhsT=wal[ct][:], rhs=xo[ct][:],
                             start=(ct == 0), stop=(ct == 1))
        al_sb = big.tile([HEADS, NB], dt.float32, tag="al_sb")
        bl_b = big.tile([HEADS, NB], dt.bfloat16, tag="bl_b")
        al_b = big.tile([HEADS, NB], dt.bfloat16, tag="al_b")
        nc.scalar.activation(al_sb[:], elp[:], Act.Exp)
        nc.scalar.activation(bl_b[:], elp[:], Act.Exp, scale=0.2)
        nc.vector.tensor_copy(al_b[:], al_sb[:])
        # drhs rows at the same row-group bases: 32h = bl_h, 32h+1 = al_h
        for h in range(HEADS):
            nc.gpsimd.dma_start(out=drhs[32 * h:32 * h + 1, :],
                                in_=bl_b[h:h + 1, :])
            nc.gpsimd.dma_start(out=drhs[32 * h + 1:32 * h + 2, :],
                                in_=al_b[h:h + 1, :])
        # al_colsT[p, it*HEADS+h] = al_sb[h, it*128+p]
        for it in range(IT):
            for h in range(HEADS):
                nc.gpsimd.dma_start(
                    out=al_colsT[:, it * HEADS + h:it * HEADS + h + 1],
                    in_=al_sb[h:h + 1, it * 128:(it + 1) * 128])

    # ---- P1 (flipped agg of the A-part): out1[i, (h,c)] accumulators --------
    # arh (ar-scaled h65, denominator rides the ones column) is produced
    # just-in-time on DVE/ACT while the PE runs the accumulation matmuls.
    p1sb = []
    with tc.tile_pool(name="pf", bufs=1, space="PSUM") as pf:
        po1f = [pf.tile([128, HEADS * C65], dt.float32, name=f"po1f_{it}",
                        tag=f"po1f_{it}") for it in range(IT)]

        def emit_arh(jt):
            a = big.tile([128, HEADS * C65], dt.bfloat16, tag=f"arh_{jt}",
                         name=f"arh_{jt}")
            ndve = 3 if jt % 2 == 0 else 2
            for h in range(HEADS):
                sc = ar_pack[:, h * JT + jt:h * JT + jt + 1]
                if h < ndve:
                    nc.vector.tensor_scalar_mul(
                        a[:, h * C65:(h + 1) * C65],
                        ht[jt][:, h * C65:(h + 1) * C65], sc)
                else:
                    nc.scalar.activation(
                        a[:, h * C65:(h + 1) * C65],
                        ht[jt][:, h * C65:(h + 1) * C65], Act.Copy, scale=sc)
            return a

        # produce arh two steps ahead of the consuming matmuls so the PE
        # queue never waits on the DVE/ACT scaling ops
        arh_q = [emit_arh(0), emit_arh(1)]
        for jt in range(JT):
            a = arh_q.pop(0)
            if jt + 2 < JT:
                arh_q.append(emit_arh(jt + 2))
            for it in range(IT):
                nc.tensor.matmul(
                    po1f[it][:], lhsT=adjT[jt][:, it * 128:(it + 1) * 128],
                    rhs=a[:], start=(jt == 0), stop=(jt == JT - 1),
                )
        for it in range(IT):
            t = big.tile([128, HEADS * C65], dt.float32, tag=f"p1sb_{it}")
            if it % 2 == 0:
                nc.scalar.copy(t[:], po1f[it][:])
            else:
                nc.vector.tensor_copy(t[:], po1f[it][:])
            p1sb.append(t)

    # ---- main loop: two head-passes so every PSUM tile double-buffers -----
    # Per pass, heads (hX, hY): d rank-2 matmuls in two concurrent PE row
    # groups, one bank each, bufs=2 -> the PE free-runs ahead of the
    # elementwise engines.  hY: fused relu+mask STT on DVE straight from
    # PSUM.  hX: ScalarE relu, then the mask (2/3 of tiles gpsimd bf16
    # multiply, 1/3 DVE int32 AND).  hX aggregation is deferred two
    # iterations so the PE never waits on the slower mask engines.
    osb2 = [None] * HEADS
    with tc.tile_pool(name="dps", bufs=2, space="PSUM") as dps, \
         tc.tile_pool(name="pacc", bufs=1, space="PSUM") as pacc:
        for hp in range(2):
            hX, hY = 2 * hp, 2 * hp + 1
            po2X = pacc.tile([C65, NB], dt.float32, name=f"po2_{hX}",
                             tag=f"po2_{hX}")
            po2Y = pacc.tile([C65, NB], dt.float32, name=f"po2_{hY}",
                             tag=f"po2_{hY}")

            def emit_d(jt):
                dA = dps.tile([128, NB], dt.float32, tag="dA")
                dB = dps.tile([128, NB], dt.float32, tag="dB")
                for h, dst in ((hX, dA), (hY, dB)):
                    nc.tensor.matmul(
                        dst[:],
                        lhsT=arbr[32 * h:32 * h + 2, jt * 128:(jt + 1) * 128],
                        rhs=drhs[32 * h:32 * h + 2, :],
                        start=True, stop=True,
                        tile_position=(32 * h, 0),
                    )
                return dA, dB

            dq = [emit_d(0), emit_d(1)]
            rq = []
            for jt in range(JT):
                dA, dB = dq.pop(0)
                r = work.tile([128, 2 * NB], dt.bfloat16, tag="r")
                rd = work2.tile([128, NB], dt.bfloat16, tag="rd")
                nc.vector.scalar_tensor_tensor(
                    out=r[:, NB:2 * NB], in0=dB[:], scalar=0.0,
                    in1=adjT[jt], op0=Alu.max, op1=Alu.mult)
                nc.scalar.activation(rd[:], dA[:], Act.Relu)
                if jt % 3 != 2:
                    nc.gpsimd.tensor_mul(r[:, 0:NB], rd[:], adjT[jt])
                else:
                    nc.vector.tensor_tensor(
                        r[:, 0:NB].bitcast(dt.int32), rd[:].bitcast(dt.int32),
                        adjm[jt], op=Alu.bitwise_and)
                if jt + 2 < JT:
                    dq.append(emit_d(jt + 2))
                nc.tensor.matmul(
                    po2Y[:], lhsT=ht[jt][:, hY * C65:(hY + 1) * C65],
                    rhs=r[:, NB:2 * NB],
                    start=(jt == 0), stop=(jt == JT - 1),
                )
                rq.append(r)
                if jt > 1:
                    nc.tensor.matmul(
                        po2X[:], lhsT=ht[jt - 2][:, hX * C65:(hX + 1) * C65],
                        rhs=rq[jt - 2][:, 0:NB],
                        start=(jt - 2 == 0), stop=False,
                    )
            for jt in (JT - 2, JT - 1):
                nc.tensor.matmul(
                    po2X[:], lhsT=ht[jt][:, hX * C65:(hX + 1) * C65],
                    rhs=rq[jt][:, 0:NB], start=False, stop=(jt == JT - 1),
                )
            tX = tr.tile([C65, NB], dt.float32, name=f"osb2_{hX}",
                         tag=f"osb2_{hX}")
            nc.scalar.copy(tX[:], po2X[:])
            osb2[hX] = tX
            tY = tr.tile([C65, NB], dt.float32, name=f"osb2_{hY}",
                         tag=f"osb2_{hY}")
            nc.vector.tensor_copy(tY[:], po2Y[:])
            osb2[hY] = tY

    # ---- epilogue: transpose the residual, combine with flipped P1 ----------
    with tc.tile_pool(name="pep", bufs=4, space="PSUM") as pep:
        for it in range(IT):
            ot = tr.tile([128, HC], dt.float32, tag="ot")
            pts = []
            for h in range(HEADS):
                pt = pep.tile([128, C65], dt.float32, tag="pt")
                nc.tensor.transpose(
                    pt[:], osb2[h][:, it * 128:(it + 1) * 128],
                    idf[0:C65, 0:C65]
                )
                pts.append(pt)
            for h in range(HEADS):
                alc = al_colsT[:, it * HEADS + h:it * HEADS + h + 1]
                # numerator cols 0..63 and the denominator col 64 in one op
                nd = tr.tile([128, C65], dt.float32, tag="nd")
                nc.vector.scalar_tensor_tensor(
                    out=nd[:], in0=p1sb[it][:, h * C65:(h + 1) * C65],
                    scalar=alc, in1=pts[h][:], op0=Alu.mult, op1=Alu.add,
                )
                rec = tr.tile([128, 1], dt.float32, tag="rec")
                nc.vector.reciprocal(rec[:], nd[:, OUT_C:C65])
                nc.vector.scalar_tensor_tensor(
                    out=ot[:, h * OUT_C:(h + 1) * OUT_C], in0=nd[:, 0:OUT_C],
                    scalar=rec[:], in1=bias_b[:, h * OUT_C:(h + 1) * OUT_C],
                    op0=Alu.mult, op1=Alu.add,
                )
            nc.sync.dma_start(out=out[it * 128:(it + 1) * 128, :], in_=ot[:])


def build():
    from contextlib import ExitStack
    import concourse.bacc as bacc
    import concourse.tile as tile
    from concourse import mybir

    dt = mybir.dt
    nc = bacc.Bacc("TRN2", target_bir_lowering=False, debug=False,
                   num_devices=NCORES)
    io = {
        "xT": nc.dram_tensor("xT", [IN_C, N], dt.bfloat16, kind="ExternalInput").ap(),
        "xoT": nc.dram_tensor("xoT", [IN_C, NB], dt.bfloat16, kind="ExternalInput").ap(),
        "adjbT": nc.dram_tensor("adjbT", [N, NB], dt.bfloat16, kind="ExternalInput").ap(),
        "adjmask": nc.dram_tensor("adjmask", [N, NB // 2], dt.int32, kind="ExternalInput").ap(),
        "Waug": nc.dram_tensor("Waug", [IN_C, WC], dt.bfloat16, kind="ExternalInput").ap(),
        "Wal": nc.dram_tensor("Wal", [IN_C, HEADS], dt.bfloat16, kind="ExternalInput").ap(),
        "bias": nc.dram_tensor("bias", [HC], dt.float32, kind="ExternalInput").ap(),
        "out": nc.dram_tensor("out", [NB, HC], dt.float32, kind="ExternalOutput").ap(),
    }
    with tile.TileContext(nc) as tc:
        with ExitStack() as ctx:
            _emit(ctx, tc, nc, io)
    nc.compile()
    return nc


def make_in_maps(x, adj, W, att_l, att_r, bias):
    import ml_dtypes
    bf16 = ml_dtypes.bfloat16
    x = np.asarray(x, np.float32)
    adj = np.asarray(adj, np.int32)
    W = np.asarray(W, np.float32)
    att_l = np.asarray(att_l, np.float32)
    att_r = np.asarray(att_r, np.float32)
    bias = np.asarray(bias, np.float32)
    xT_b = np.ascontiguousarray(x.T.astype(bf16))
    Wr = W.reshape(IN_C, HEADS, OUT_C)
    Wal_ = np.ascontiguousarray(
        np.einsum("khc,hc->kh", Wr, att_l).astype(bf16))
    War = np.einsum("khc,hc->kh", Wr, att_r)
    Waug_b = np.ascontiguousarray(
        np.concatenate([W, War], axis=1).astype(bf16))
    adj_b = adj.astype(bf16)
    in_maps = []
    for m in range(NCORES):
        sl = slice(m * NB, (m + 1) * NB)
        adjbT = np.ascontiguousarray(adj_b[sl].T)
        mask16 = np.where(adjbT != 0, np.uint16(0xFFFF), np.uint16(0))
        adjmask = np.ascontiguousarray(mask16).view(np.int32)
        in_maps.append({
            "xT": xT_b,
            "xoT": np.ascontiguousarray(x[sl].T.astype(bf16)),
            "adjbT": adjbT,
            "adjmask": adjmask,
            "Waug": Waug_b,
            "Wal": Wal_,
            "bias": bias,
        })
    return in_maps


def _install_ntff_shim():
    # this container image lacks antenv.axon_hooks; recreate it from the boot
    # helper so run_bass_kernel_spmd's trace path can find the profile hook
    import sys, types
    if "antenv.axon_hooks" in sys.modules:
        return
    from trn_agent_boot.trn_boot import _ntff_profile_via_ctypes
    hook = _ntff_profile_via_ctypes("/opt/axon/libaxon_pjrt.so")
    mod = types.ModuleType("antenv.axon_hooks")
    mod.get_axon_ntff_profile_hook = lambda: hook
    mod.set_axon_ntff_profile_hook = lambda h: None
    sys.modules["antenv.axon_hooks"] = mod


def kernel(x, adj, W, att_l, att_r, bias):
    from concourse.bass_utils import run_bass_kernel_spmd

    if "nc" not in _compiled:
        _compiled["nc"] = build()
    nc = _compiled["nc"]
    in_maps = make_in_maps(x, adj, W, att_l, att_r, bias)
    kwargs = {}
    if TRACE:
        _install_ntff_shim()
        kwargs["trace"] = True
    res = run_bass_kernel_spmd(nc, in_maps, core_ids=list(range(NCORES)), **kwargs)
    LAST_RESULTS["exec_time_ns"] = res.exec_time_ns
    LAST_RESULTS["mean_exec_time_ns"] = res.mean_exec_time_ns
    LAST_RESULTS["res"] = res
    return np.concatenate([res.results[m]["out"] for m in range(NCORES)], axis=0)
